# revision 15
# baseline (speedup 1.0000x reference)
"""BitLinear (fake-quant straight-through) Trainium2 kernel.

Math (per the reference nn module):
  dqx = round(x * s_x) / s_x         s_x = 127 / clip(rowabsmax(x), 1e-5)   (per token row)
  dqw = clip(round(w * s_w), -1, 1) / s_w    s_w = 1 / clip(mean(|w|), 1e-5)  (per tensor)
  out = dqx @ dqw.T + bias

Key facts this kernel exploits:
  * round(x*s_x) is an integer in [-127, 127] and clip(round(w*s_w)) is in
    {-1, 0, 1}; both are EXACT in bf16, and the matmul accumulates in fp32
    PSUM where all partial sums (<= 2^17) are exact integers.  So the heavy
    matmul runs at bf16 PE rate with zero quantization-path error; the
    per-token / per-tensor scales are applied to the (exact) integer matmul
    result afterwards.
  * round-half-even == fp32 RNE, so `round(v)` is computed exactly as
    `(v + 1.5*2^23) - 1.5*2^23` with two fp32 ALU stages (no Round op needed).

Sharding: data parallel over the batch dim; core i computes batch element i
with the full weight.  No collectives; the host scatters x and gathers out.

The per-tensor weight scale s_w is the one input-derived scalar computed on
the host (it must match the reference's fp32 mean reduction to ~1 ulp, which
an on-device sequential reduction cannot guarantee; a 1e-6 relative error in
s_w flips ternary weights and produces visible output error).  It is passed
in through a small constants tensor, so the compiled program is input-
independent.
"""

import numpy as np

from concourse import bacc, bass, mybir, tile
from concourse.bass_utils import run_bass_kernel_spmd

F32 = mybir.dt.float32
BF16 = mybir.dt.bfloat16
ALU = mybir.AluOpType
ACTF = mybir.ActivationFunctionType

MAGIC = 12582912.0  # 1.5 * 2**23: fp32 RNE round-to-integer constant
EPS = 1e-05

B, S, K, N = 8, 4096, 1024, 1024
N_CORES = 8


def build(s_tokens=S, k=K, n=N):
    """Build the single-core SPMD program: x[s_tokens,k] @ w[n,k]^T quantized."""
    nc = bacc.Bacc("TRN2", target_bir_lowering=False, debug=False)

    x_d = nc.dram_tensor("x", [s_tokens, k], F32, kind="ExternalInput").ap()
    w_d = nc.dram_tensor("w", [n, k], F32, kind="ExternalInput").ap()
    biasb_d = nc.dram_tensor("biasb", [128, n], F32, kind="ExternalInput").ap()
    consts_d = nc.dram_tensor("consts", [128, 8], F32, kind="ExternalInput").ap()
    out_d = nc.dram_tensor("out", [s_tokens, n], F32, kind="ExternalOutput").ap()

    ST = s_tokens // 128  # token tiles
    KT = k // 128         # contraction tiles
    NT = n // 128         # weight row tiles
    NH = n // 512         # psum-bank halves of the output feature dim

    x_t = x_d.rearrange("(t p) k -> t p k", p=128)
    out_t = out_d.rearrange("(t p) n -> t p n", p=128)
    w_t = w_d.rearrange("(t p) k -> t p k", p=128)

    with tile.TileContext(nc) as tc:
        with (
            tc.tile_pool(name="static", bufs=1) as static,
            tc.tile_pool(name="wstage", bufs=2) as wstage,
            tc.tile_pool(name="xpool", bufs=6) as xpool,
            tc.tile_pool(name="tpool", bufs=6) as tpool,
            tc.tile_pool(name="qpool", bufs=6) as qpool,
            tc.tile_pool(name="qtpool", bufs=6) as qtpool,
            tc.tile_pool(name="opool", bufs=6) as opool,
            tc.tile_pool(name="vpool", bufs=8) as vpool,
            tc.tile_pool(name="psum", bufs=3, space="PSUM") as psum_pool,
        ):
            consts = static.tile([128, 8], F32)
            nc.sync.dma_start(consts[:], consts_d[:])
            biasb = static.tile([128, n], F32)
            nc.sync.dma_start(biasb[:], biasb_d[:])
            # qwT[kpart, kt, n] = quantized weight, transposed: k on partitions
            qwT = static.tile([128, KT, n], BF16)

            sw_ap = consts[:, 0:1]    # s_w  (weight quant scale)
            c127 = consts[:, 1:2]     # 127.0
            k1 = consts[:, 2:3]       # (1/s_w) / 127  (output scale factor)

            # ---- weight quantization + transpose (one-time) ----
            for nt in range(NT):
                w_s = wstage.tile([128, k], F32, name="w_s")
                nc.sync.dma_start(w_s[:], w_t[nt])
                # fl(w*s_w) then fl(+MAGIC): RNE round-to-int, matching
                # the reference's separate mul-then-round rounding points.
                tw = wstage.tile([128, k], F32, name="tw")
                nc.vector.tensor_scalar(tw[:], w_s[:], sw_ap, MAGIC, ALU.mult, ALU.add)
                qwf = wstage.tile([128, k], F32, name="qwf")
                nc.vector.tensor_scalar_sub(qwf[:], tw[:], MAGIC)
                qw = wstage.tile([128, k], BF16, name="qw")
                nc.vector.tensor_scalar(qw[:], qwf[:], 1.0, -1.0, ALU.min, ALU.max)
                # one batched xbar transpose per n-tile: [128n, 1024k] ->
                # [128k, KT, 128n] (out row r = j*128+p, verified in sim)
                nc.sync.dma_start_transpose(
                    qwT[:, :, nt * 128:(nt + 1) * 128], qw[:]
                )

            # ---- main pipeline over token tiles ----
            LEAD = 5
            x_tiles = {}

            def issue_load(tt):
                x_s = xpool.tile([128, k], F32, name="x_s")
                nc.sync.dma_start(x_s[:], x_t[tt])
                x_tiles[tt] = x_s

            for j in range(min(LEAD, ST)):
                issue_load(j)

            for t in range(ST):
                if t + LEAD < ST:
                    issue_load(t + LEAD)
                x_s = x_tiles.pop(t)

                c = vpool.tile([128, 1], F32, name="c")
                nc.vector.tensor_reduce(
                    c[:], x_s[:], mybir.AxisListType.X, ALU.max,
                    apply_absolute_value=True,
                )
                cc = vpool.tile([128, 1], F32, name="cc")
                nc.vector.tensor_scalar_max(cc[:], c[:], EPS)
                rc = vpool.tile([128, 1], F32, name="rc")
                nc.vector.reciprocal(rc[:], cc[:])
                ss = vpool.tile([128, 1], F32, name="ss")
                nc.vector.tensor_scalar_mul(ss[:], rc[:], 127.0)
                fs = vpool.tile([128, 1], F32, name="fs")
                nc.vector.tensor_scalar_mul(fs[:], cc[:], k1)

                tq = tpool.tile([128, k], F32, name="tq")
                nc.scalar.activation(tq[:], x_s[:], ACTF.Copy, bias=MAGIC, scale=ss[:])
                qx = qpool.tile([128, k], BF16, name="qx")
                nc.vector.tensor_scalar_sub(qx[:], tq[:], MAGIC)

                qxT = qtpool.tile([128, KT, 128], BF16, name="qxT")
                nc.sync.dma_start_transpose(qxT[:], qx[:])

                outs = opool.tile([128, n], F32, name="outs")
                for h in range(NH):
                    ps = psum_pool.tile([128, 512], F32, name=f"ps{h}", tag=f"ps{h}")
                    for kt in range(KT):
                        nc.tensor.matmul(
                            ps[:],
                            qxT[:, kt, :],
                            qwT[:, kt, h * 512:(h + 1) * 512],
                            start=(kt == 0),
                            stop=(kt == KT - 1),
                        )
                    nc.scalar.activation(
                        outs[:, h * 512:(h + 1) * 512], ps[:], ACTF.Copy, scale=fs[:]
                    )
                nc.gpsimd.tensor_tensor(outs[:], outs[:], biasb[:], ALU.add)
                nc.gpsimd.dma_start(out_t[t], outs[:])

    nc.compile()
    return nc


def host_consts(weight):
    """The one input-derived scalar: s_w, matching the reference's fp32 mean."""
    try:
        import jax
        import jax.numpy as jnp

        with jax.default_device(jax.devices("cpu")[0]):
            mean_abs = np.float32(
                jax.device_get(jnp.mean(jnp.abs(jnp.asarray(weight, dtype=jnp.float32))))
            )
    except Exception:
        mean_abs = np.float32(np.mean(np.abs(weight), dtype=np.float32))
    mean_c = np.maximum(mean_abs, np.float32(EPS))
    sw = np.float32(1.0) / mean_c          # s_w, the weight quant scale
    wdiv = np.float32(1.0) / sw            # 1/s_w (the ternary unit value)
    k1 = wdiv / np.float32(127.0)          # output scale = cc * k1
    row = np.zeros((8,), np.float32)
    row[0], row[1], row[2] = sw, np.float32(127.0), k1
    return np.tile(row[None, :], (128, 1)).copy()


_NC_CACHE = {}


def _get_nc():
    if "nc" not in _NC_CACHE:
        _NC_CACHE["nc"] = build()
    return _NC_CACHE["nc"]


def make_in_maps(x, weight, bias):
    x = np.ascontiguousarray(x, dtype=np.float32)
    weight = np.ascontiguousarray(weight, dtype=np.float32)
    bias = np.ascontiguousarray(bias, dtype=np.float32)
    consts = host_consts(weight)
    biasb = np.ascontiguousarray(np.broadcast_to(bias[None, :], (128, N)))
    return [
        {"x": x[i], "w": weight, "biasb": biasb, "consts": consts}
        for i in range(N_CORES)
    ]


def kernel(x, weight, bias, **kwargs):
    nc = _get_nc()
    in_maps = make_in_maps(x, weight, bias)
    res = run_bass_kernel_spmd(nc, in_maps, list(range(N_CORES)))
    return np.stack([res.results[i]["out"] for i in range(N_CORES)], axis=0)


# revision 17
# speedup vs baseline: 1.0885x; 1.0885x over previous
"""BitLinear (fake-quant straight-through) Trainium2 kernel.

Math (per the reference nn module):
  dqx = round(x * s_x) / s_x         s_x = 127 / clip(rowabsmax(x), 1e-5)   (per token row)
  dqw = clip(round(w * s_w), -1, 1) / s_w    s_w = 1 / clip(mean(|w|), 1e-5)  (per tensor)
  out = dqx @ dqw.T + bias

Key facts this kernel exploits:
  * round(x*s_x) is an integer in [-127, 127] and clip(round(w*s_w)) is in
    {-1, 0, 1}; both are EXACT in bf16, and the matmul accumulates in fp32
    PSUM where all partial sums (<= 2^17) are exact integers.  So the heavy
    matmul runs at bf16 PE rate with zero quantization-path error; the
    per-token / per-tensor scales are applied to the (exact) integer matmul
    result afterwards.
  * round-half-even == fp32 RNE, so `round(v)` is computed exactly as
    `(v + 1.5*2^23) - 1.5*2^23` with two fp32 ALU stages (no Round op needed).

Sharding: data parallel over the batch dim; core i computes batch element i
with the full weight.  No collectives; the host scatters x and gathers out.

The per-tensor weight scale s_w is the one input-derived scalar computed on
the host (it must match the reference's fp32 mean reduction to ~1 ulp, which
an on-device sequential reduction cannot guarantee; a 1e-6 relative error in
s_w flips ternary weights and produces visible output error).  It is passed
in through a small constants tensor, so the compiled program is input-
independent.
"""

import numpy as np

from concourse import bacc, bass, mybir, tile
from concourse.bass_utils import run_bass_kernel_spmd

F32 = mybir.dt.float32
BF16 = mybir.dt.bfloat16
ALU = mybir.AluOpType
ACTF = mybir.ActivationFunctionType

MAGIC = 12582912.0  # 1.5 * 2**23: fp32 RNE round-to-integer constant
EPS = 1e-05

B, S, K, N = 8, 4096, 1024, 1024
N_CORES = 8


def build(s_tokens=S, k=K, n=N):
    """Build the single-core SPMD program: x[s_tokens,k] @ w[n,k]^T quantized."""
    nc = bacc.Bacc("TRN2", target_bir_lowering=False, debug=False)

    x_d = nc.dram_tensor("x", [s_tokens, k], F32, kind="ExternalInput").ap()
    w_d = nc.dram_tensor("w", [n, k], F32, kind="ExternalInput").ap()
    biasb_d = nc.dram_tensor("biasb", [128, n], F32, kind="ExternalInput").ap()
    consts_d = nc.dram_tensor("consts", [128, 8], F32, kind="ExternalInput").ap()
    out_d = nc.dram_tensor("out", [s_tokens, n], F32, kind="ExternalOutput").ap()

    ST = s_tokens // 128  # token tiles
    KT = k // 128         # contraction tiles
    NT = n // 128         # weight row tiles
    NH = n // 512         # psum-bank halves of the output feature dim

    x_t = x_d.rearrange("(t p) k -> t p k", p=128)
    out_t = out_d.rearrange("(t p) n -> t p n", p=128)
    w_t = w_d.rearrange("(t p) k -> t p k", p=128)

    with tile.TileContext(nc) as tc:
        with (
            tc.tile_pool(name="static", bufs=1) as static,
            tc.tile_pool(name="wstage", bufs=2) as wstage,
            tc.tile_pool(name="xpool", bufs=6) as xpool,
            tc.tile_pool(name="tpool", bufs=6) as tpool,
            tc.tile_pool(name="qpool", bufs=6) as qpool,
            tc.tile_pool(name="qtpool", bufs=6) as qtpool,
            tc.tile_pool(name="opool", bufs=6) as opool,
            tc.tile_pool(name="vpool", bufs=8) as vpool,
            tc.tile_pool(name="psum", bufs=3, space="PSUM") as psum_pool,
        ):
            consts = static.tile([128, 8], F32)
            nc.sync.dma_start(consts[:], consts_d[:])
            biasb = static.tile([128, n], F32)
            nc.sync.dma_start(biasb[:], biasb_d[:])
            # qwT[kpart, kt, n] = quantized weight, transposed: k on partitions
            qwT = static.tile([128, KT, n], BF16)

            sw_ap = consts[:, 0:1]    # s_w  (weight quant scale)
            c127 = consts[:, 1:2]     # 127.0
            k1 = consts[:, 2:3]       # (1/s_w) / 127  (output scale factor)

            # ---- weight quantization + transpose (one-time) ----
            for nt in range(NT):
                w_s = wstage.tile([128, k], F32, name="w_s")
                nc.sync.dma_start(w_s[:], w_t[nt])
                # fl(w*s_w) then fl(+MAGIC): RNE round-to-int, matching
                # the reference's separate mul-then-round rounding points.
                tw = wstage.tile([128, k], F32, name="tw")
                nc.vector.tensor_scalar(tw[:], w_s[:], sw_ap, MAGIC, ALU.mult, ALU.add)
                qwf = wstage.tile([128, k], F32, name="qwf")
                nc.vector.tensor_scalar_sub(qwf[:], tw[:], MAGIC)
                qw = wstage.tile([128, k], BF16, name="qw")
                nc.vector.tensor_scalar(qw[:], qwf[:], 1.0, -1.0, ALU.min, ALU.max)
                # one batched xbar transpose per n-tile: [128n, 1024k] ->
                # [128k, KT, 128n] (out row r = j*128+p, verified in sim)
                nc.sync.dma_start_transpose(
                    qwT[:, :, nt * 128:(nt + 1) * 128], qw[:]
                )

            # ---- main pipeline over token tiles ----
            for t in range(ST):
                x_s = xpool.tile([128, k], F32, name="x_s")
                nc.sync.dma_start(x_s[:], x_t[t])

                c = vpool.tile([128, 1], F32, name="c")
                nc.vector.tensor_reduce(
                    c[:], x_s[:], mybir.AxisListType.X, ALU.max,
                    apply_absolute_value=True,
                )
                cc = vpool.tile([128, 1], F32, name="cc")
                nc.vector.tensor_scalar_max(cc[:], c[:], EPS)
                rc = vpool.tile([128, 1], F32, name="rc")
                nc.vector.reciprocal(rc[:], cc[:])
                ss = vpool.tile([128, 1], F32, name="ss")
                nc.vector.tensor_scalar_mul(ss[:], rc[:], 127.0)
                fs = vpool.tile([128, 1], F32, name="fs")
                nc.vector.tensor_scalar_mul(fs[:], cc[:], k1)

                tq = tpool.tile([128, k], F32, name="tq")
                nc.vector.tensor_scalar(tq[:], x_s[:], ss[:], MAGIC, ALU.mult, ALU.add)
                qx = qpool.tile([128, k], BF16, name="qx")
                nc.vector.tensor_scalar_sub(qx[:], tq[:], MAGIC)

                qxT = qtpool.tile([128, KT, 128], BF16, name="qxT")
                nc.sync.dma_start_transpose(qxT[:], qx[:])

                outs = opool.tile([128, n], F32, name="outs")
                for h in range(NH):
                    ps = psum_pool.tile([128, 512], F32, name=f"ps{h}", tag=f"ps{h}")
                    for kt in range(KT):
                        nc.tensor.matmul(
                            ps[:],
                            qxT[:, kt, :],
                            qwT[:, kt, h * 512:(h + 1) * 512],
                            start=(kt == 0),
                            stop=(kt == KT - 1),
                        )
                    nc.scalar.activation(
                        outs[:, h * 512:(h + 1) * 512], ps[:], ACTF.Copy, scale=fs[:]
                    )
                nc.gpsimd.tensor_tensor(outs[:], outs[:], biasb[:], ALU.add)
                nc.gpsimd.dma_start(out_t[t], outs[:])

    nc.compile()
    return nc


def host_consts(weight):
    """The one input-derived scalar: s_w, matching the reference's fp32 mean."""
    try:
        import jax
        import jax.numpy as jnp

        with jax.default_device(jax.devices("cpu")[0]):
            mean_abs = np.float32(
                jax.device_get(jnp.mean(jnp.abs(jnp.asarray(weight, dtype=jnp.float32))))
            )
    except Exception:
        mean_abs = np.float32(np.mean(np.abs(weight), dtype=np.float32))
    mean_c = np.maximum(mean_abs, np.float32(EPS))
    sw = np.float32(1.0) / mean_c          # s_w, the weight quant scale
    wdiv = np.float32(1.0) / sw            # 1/s_w (the ternary unit value)
    k1 = wdiv / np.float32(127.0)          # output scale = cc * k1
    row = np.zeros((8,), np.float32)
    row[0], row[1], row[2] = sw, np.float32(127.0), k1
    return np.tile(row[None, :], (128, 1)).copy()


_NC_CACHE = {}


def _get_nc():
    if "nc" not in _NC_CACHE:
        _NC_CACHE["nc"] = build()
    return _NC_CACHE["nc"]


def make_in_maps(x, weight, bias):
    x = np.ascontiguousarray(x, dtype=np.float32)
    weight = np.ascontiguousarray(weight, dtype=np.float32)
    bias = np.ascontiguousarray(bias, dtype=np.float32)
    consts = host_consts(weight)
    biasb = np.ascontiguousarray(np.broadcast_to(bias[None, :], (128, N)))
    return [
        {"x": x[i], "w": weight, "biasb": biasb, "consts": consts}
        for i in range(N_CORES)
    ]


def kernel(x, weight, bias, **kwargs):
    nc = _get_nc()
    in_maps = make_in_maps(x, weight, bias)
    res = run_bass_kernel_spmd(nc, in_maps, list(range(N_CORES)))
    return np.stack([res.results[i]["out"] for i in range(N_CORES)], axis=0)


# revision 18
# speedup vs baseline: 1.4547x; 1.3364x over previous
"""BitLinear (fake-quant straight-through) Trainium2 kernel.

Math (per the reference nn module):
  dqx = round(x * s_x) / s_x         s_x = 127 / clip(rowabsmax(x), 1e-5)   (per token row)
  dqw = clip(round(w * s_w), -1, 1) / s_w    s_w = 1 / clip(mean(|w|), 1e-5)  (per tensor)
  out = dqx @ dqw.T + bias

Key facts this kernel exploits:
  * round(x*s_x) is an integer in [-127, 127] and clip(round(w*s_w)) is in
    {-1, 0, 1}; both are EXACT in bf16, and the matmul accumulates in fp32
    PSUM where all partial sums (<= 2^17) are exact integers.  So the heavy
    matmul runs at bf16 PE rate with zero quantization-path error; the
    per-token / per-tensor scales are applied to the (exact) integer matmul
    result afterwards.
  * round-half-even == fp32 RNE, so `round(v)` is computed exactly as
    `(v + 1.5*2^23) - 1.5*2^23` with two fp32 ALU stages (no Round op needed).

Sharding: data parallel over the batch dim; core i computes batch element i
with the full weight.  No collectives; the host scatters x and gathers out.

Pipeline structure: tokens are processed in "quads" (4 x 128 = 512 tokens).
One 2 MiB load, one [128,4,1024] absmax reduce, one batched xbar transpose
([128, 4096]bf16 -> [128, 32, 128], row r = j*128+p), and 64 back-to-back
matmuls per quad -- amortizing the per-stage semaphore latency 4x and
keeping the PE's HAM clock-gate warm within each quad.

Engine assignment (each pipeline stage owns an engine; the PSUM-evacuation
engine (ACT) carries no upstream work so input prep never waits on matmuls):
  sync   : input DMA + xbar transposes
  vector : absmax reduce, scales, quantize (round via magic constant)
  tensor : matmuls (bf16 exact-integer)
  scalar : PSUM evacuation with per-token output scale
  gpsimd : bias add + output store (SWDGE)

The per-tensor weight scale s_w is the one input-derived scalar computed on
the host (it must match the reference's fp32 mean reduction to ~1 ulp, which
an on-device sequential reduction cannot guarantee; a 1e-6 relative error in
s_w flips ternary weights and produces visible output error).  It is passed
in through a small constants tensor, so the compiled program is input-
independent.
"""

import numpy as np

from concourse import bacc, bass, mybir, tile
from concourse.bass_utils import run_bass_kernel_spmd

F32 = mybir.dt.float32
BF16 = mybir.dt.bfloat16
ALU = mybir.AluOpType
ACTF = mybir.ActivationFunctionType

MAGIC = 12582912.0  # 1.5 * 2**23: fp32 RNE round-to-integer constant
EPS = 1e-05

B, S, K, N = 8, 4096, 1024, 1024
N_CORES = 8
QS = 4  # token tiles per quad


def build(s_tokens=S, k=K, n=N):
    """Build the single-core SPMD program: x[s_tokens,k] @ w[n,k]^T quantized."""
    nc = bacc.Bacc("TRN2", target_bir_lowering=False, debug=False)

    x_d = nc.dram_tensor("x", [s_tokens, k], F32, kind="ExternalInput").ap()
    w_d = nc.dram_tensor("w", [n, k], F32, kind="ExternalInput").ap()
    biasb_d = nc.dram_tensor("biasb", [128, QS, n], F32, kind="ExternalInput").ap()
    consts_d = nc.dram_tensor("consts", [128, 8], F32, kind="ExternalInput").ap()
    out_d = nc.dram_tensor("out", [s_tokens, n], F32, kind="ExternalOutput").ap()

    KT = k // 128          # contraction tiles
    NT = n // 128          # weight row tiles
    NH = n // 512          # psum-bank halves of the output feature dim
    NQ = s_tokens // (128 * QS)  # quads

    x_q = x_d.rearrange("(q s p) k -> q p s k", s=QS, p=128)
    out_q = out_d.rearrange("(q s p) n -> q p s n", s=QS, p=128)
    w_t = w_d.rearrange("(t p) k -> t p k", p=128)

    with tile.TileContext(nc) as tc:
        with (
            tc.tile_pool(name="static", bufs=1) as static,
            tc.tile_pool(name="wstage", bufs=2) as wstage,
            tc.tile_pool(name="xpool", bufs=2) as xpool,
            tc.tile_pool(name="qpool", bufs=2) as qpool,
            tc.tile_pool(name="qtpool", bufs=2) as qtpool,
            tc.tile_pool(name="opool", bufs=2) as opool,
            tc.tile_pool(name="vpool", bufs=4) as vpool,
            tc.tile_pool(name="psum", bufs=3, space="PSUM") as psum_pool,
        ):
            consts = static.tile([128, 8], F32)
            nc.sync.dma_start(consts[:], consts_d[:])
            biasb = static.tile([128, QS, n], F32)
            nc.sync.dma_start(biasb[:], biasb_d[:])
            # qwT[kpart, kt, n] = quantized weight, transposed: k on partitions
            qwT = static.tile([128, KT, n], BF16)

            sw_ap = consts[:, 0:1]    # s_w  (weight quant scale)
            k1 = consts[:, 2:3]       # (1/s_w) / 127  (output scale factor)

            # ---- weight quantization + transpose (one-time) ----
            for nt in range(NT):
                w_s = wstage.tile([128, k], F32, name="w_s")
                nc.sync.dma_start(w_s[:], w_t[nt])
                # fl(w*s_w) then fl(+MAGIC): RNE round-to-int, matching
                # the reference's separate mul-then-round rounding points.
                tw = wstage.tile([128, k], F32, name="tw")
                nc.vector.tensor_scalar(tw[:], w_s[:], sw_ap, MAGIC, ALU.mult, ALU.add)
                qwf = wstage.tile([128, k], F32, name="qwf")
                nc.vector.tensor_scalar_sub(qwf[:], tw[:], MAGIC)
                qw = wstage.tile([128, k], BF16, name="qw")
                nc.vector.tensor_scalar(qw[:], qwf[:], 1.0, -1.0, ALU.min, ALU.max)
                # one batched xbar transpose per n-tile: [128n, 1024k] ->
                # [128k, KT, 128n] (out row r = j*128+p, verified on hw)
                nc.sync.dma_start_transpose(
                    qwT[:, :, nt * 128:(nt + 1) * 128], qw[:]
                )

            # ---- main pipeline over token quads ----
            for q in range(NQ):
                x_s = xpool.tile([128, QS, k], F32, name="x_s")
                nc.sync.dma_start(x_s[:], x_q[q])

                c = vpool.tile([128, QS], F32, name="c")
                nc.vector.tensor_reduce(
                    c[:], x_s[:], mybir.AxisListType.X, ALU.max,
                    apply_absolute_value=True,
                )
                cc = vpool.tile([128, QS], F32, name="cc")
                nc.vector.tensor_scalar_max(cc[:], c[:], EPS)
                rc = vpool.tile([128, QS], F32, name="rc")
                nc.vector.reciprocal(rc[:], cc[:])
                ss = vpool.tile([128, QS], F32, name="ss")
                nc.vector.tensor_scalar_mul(ss[:], rc[:], 127.0)
                fs = vpool.tile([128, QS], F32, name="fs")
                nc.vector.tensor_scalar_mul(fs[:], cc[:], k1)

                # round(x*s_x) via magic constant, in place on x_s, then to bf16
                for s in range(QS):
                    nc.vector.tensor_scalar(
                        x_s[:, s, :], x_s[:, s, :], ss[:, s:s + 1], MAGIC,
                        ALU.mult, ALU.add,
                    )
                qx = qpool.tile([128, QS, k], BF16, name="qx")
                nc.vector.tensor_scalar_sub(qx[:], x_s[:], MAGIC)

                # one xbar transpose for the whole quad:
                # [128s, QS*k] -> [128k, QS*KT, 128s], chunk j = s*KT + kt
                qxT = qtpool.tile([128, QS, KT, 128], BF16, name="qxT")
                nc.sync.dma_start_transpose(qxT[:], qx[:])

                outs = opool.tile([128, QS, n], F32, name="outs")
                for s in range(QS):
                    ps_list = [
                        psum_pool.tile([128, 512], F32, name=f"ps{h}", tag=f"ps{h}")
                        for h in range(NH)
                    ]
                    for kt in range(KT):
                        for h in range(NH):
                            nc.tensor.matmul(
                                ps_list[h][:],
                                qxT[:, s, kt, :],
                                qwT[:, kt, h * 512:(h + 1) * 512],
                                start=(kt == 0),
                                stop=(kt == KT - 1),
                            )
                    for h in range(NH):
                        nc.scalar.activation(
                            outs[:, s, h * 512:(h + 1) * 512], ps_list[h][:],
                            ACTF.Copy, scale=fs[:, s:s + 1],
                        )
                nc.gpsimd.tensor_tensor(outs[:], outs[:], biasb[:], ALU.add)
                nc.gpsimd.dma_start(out_q[q], outs[:])

    nc.compile()
    return nc


def host_consts(weight):
    """The one input-derived scalar: s_w, matching the reference's fp32 mean."""
    try:
        import jax
        import jax.numpy as jnp

        with jax.default_device(jax.devices("cpu")[0]):
            mean_abs = np.float32(
                jax.device_get(jnp.mean(jnp.abs(jnp.asarray(weight, dtype=jnp.float32))))
            )
    except Exception:
        mean_abs = np.float32(np.mean(np.abs(weight), dtype=np.float32))
    mean_c = np.maximum(mean_abs, np.float32(EPS))
    sw = np.float32(1.0) / mean_c          # s_w, the weight quant scale
    wdiv = np.float32(1.0) / sw            # 1/s_w (the ternary unit value)
    k1 = wdiv / np.float32(127.0)          # output scale = cc * k1
    row = np.zeros((8,), np.float32)
    row[0], row[1], row[2] = sw, np.float32(127.0), k1
    return np.tile(row[None, :], (128, 1)).copy()


_NC_CACHE = {}


def _get_nc():
    if "nc" not in _NC_CACHE:
        _NC_CACHE["nc"] = build()
    return _NC_CACHE["nc"]


def make_in_maps(x, weight, bias):
    x = np.ascontiguousarray(x, dtype=np.float32)
    weight = np.ascontiguousarray(weight, dtype=np.float32)
    bias = np.ascontiguousarray(bias, dtype=np.float32)
    consts = host_consts(weight)
    biasb = np.ascontiguousarray(
        np.broadcast_to(bias[None, None, :], (128, QS, N))
    )
    return [
        {"x": x[i], "w": weight, "biasb": biasb, "consts": consts}
        for i in range(N_CORES)
    ]


def kernel(x, weight, bias, **kwargs):
    nc = _get_nc()
    in_maps = make_in_maps(x, weight, bias)
    last_err = None
    for _attempt in range(3):
        try:
            res = run_bass_kernel_spmd(nc, in_maps, list(range(N_CORES)))
            return np.stack([res.results[i]["out"] for i in range(N_CORES)], axis=0)
        except Exception as e:  # transient NRT device errors: retry
            last_err = e
    raise last_err
